# revision 1
# baseline (speedup 1.0000x reference)
"""ChebConv (order-4) GNN layer on 8 Trainium2 NeuronCores.

Reference computation (fp32):
    T0 = x, T1 = G x, Tk = 2 G T{k-1} - T{k-2}
    out = sum_k Tk @ W[k]          # [N, F] with N=10000, F=32

Strategy:
  * Rewrite in the power basis: y0 = x, yk = G y{k-1},
      out = sum_k yk @ Wp[k]  with
      Wp = [W0 - W2, W1 - 3 W3, 2 W2, 4 W3]   (exact modulo fp reassociation)
    so each hop is a bare matmul against G (no 2*/- epilogue).
  * Row-shard G over 8 cores (1280 padded rows each). The per-core lhsT
    tiles must hold G^T, so the host passes each core a contiguous
    transposed slice (pad N 10000 -> 10240).
  * fp32 matmuls on the TRN2 PE run in LOW_HIGH mode: 2 passes, each
    streaming the fp32 rhs at half rate (4x bf16 cost). Instead we do a
    software hi/lo split: G = G_hi + G_lo and v = v_hi + v_lo (bf16
    pairs) and compute G_hi v_hi + G_lo v_hi + G_hi v_lo with fp32 PSUM
    accumulation -- 3 full-rate bf16 passes, same DRAM bytes as fp32,
    ~7e-6 relative error (vs 3e-3 for plain bf16). Per fc sweep, G_hi
    and G_lo rows are interleaved in one [NP, 2*l] array so each
    128-row j-chunk is a single contiguous DMA.
  * Each hop runs as 3 sweeps, one per <=512-column chunk of yk^T.
    Per sweep and 128-row j-chunk: matmuls (lhsT=v_{hi,lo}[j-chunk]
    [128,32] bf16, rhs=G^T_{hi,lo} tile [128,<=512] bf16) accumulate
    the sweep's [32,<=512] chunk of yk^T over all 80 j-chunks (one open
    PSUM accumulation group per bank). The last (smallest) sweep's G
    block stays pinned in SBUF across hops (10.5 MB saved twice).
  * The Wp contraction happens on-chip from yk^T in full fp32:
    matmul(lhsT=Wp_k [32,32], rhs=ykT chunk), DVE-add into the
    transposed output accumulator; the k=0 term uses the host xT slice.
  * After each sweep (except in the last hop), its rows are
    PE-transposed ([32,128] -> [128,32] blocks) into natural m-chunk
    layout, split into bf16 hi/lo, and all-gathered in a partial
    collective (DRAM bounce) that overlaps the remaining sweeps. The
    reload into the next hop's per-part v tiles rides SWDGE (gpsimd) so
    the gather-gated DMA cannot convoy the G stream on the shared HWDGE
    completion lanes; j-chunks are consumed in gather-firing order so
    each hop starts on columns whose gather finished first.
  * Output is returned transposed ([32, 1280] per core); the host
    concatenates, transposes and drops padding.
"""

import sys

if "/opt/trn_rl_repo" not in sys.path:
    sys.path.insert(0, "/opt/trn_rl_repo")

import numpy as np

N = 10000
F = 32
ORDER = 4
NCORES = 8
P = 128
NP = 10240  # padded node count: divisible by NCORES * P
RPC = NP // NCORES  # rows per core (1280)
JC = NP // P  # global 128-row chunks (80)
MC = RPC // P  # local 128-row chunks per core (10)

_CACHE = {}


def _build(np_total, ncores):
    from concourse import bacc, masks, mybir, tile

    rpc = np_total // ncores
    jc = np_total // P
    mc = rpc // P
    f32 = mybir.dt.float32
    bf16 = mybir.dt.bfloat16
    fchunks = [(s, min(512, rpc - s)) for s in range(0, rpc, 512)]
    nfc = len(fchunks)

    nc = bacc.Bacc(
        "TRN2", target_bir_lowering=False, debug=False, num_devices=ncores
    )
    # one G^T block per fc sweep, rows = [hi cols | lo cols] interleaved
    ghls = [
        nc.dram_tensor(f"ghl{i}", [np_total, 2 * l], bf16, kind="ExternalInput").ap()
        for i, (s, l) in enumerate(fchunks)
    ]
    # per-part m-chunk geometry: part i covers m-chunks [m0, m0+nm)
    parts = [(s // P, l // P) for s, l in fchunks]
    # x in per-part v layout: concat over parts of [hi block | lo block],
    # block col (c*nm + ml)*F + f = padded x row (c*mc + m0 + ml)*P + p
    vcols = [2 * ncores * nm * F for (m0, nm) in parts]
    xthl = nc.dram_tensor("xthl", [P, sum(vcols)], bf16, kind="ExternalInput").ap()
    xt = nc.dram_tensor("xt", [F, rpc], f32, kind="ExternalInput").ap()
    wp = nc.dram_tensor("wp", [F, ORDER * F], f32, kind="ExternalInput").ap()
    out_t = nc.dram_tensor("outT", [F, rpc], f32, kind="ExternalOutput").ap()

    # pin the last (smallest) fc sweep's G block in SBUF across hops
    pin_i = nfc - 1
    pin_l = fchunks[pin_i][1]

    def part_of(m):
        for i, (m0, nm) in enumerate(parts):
            if m0 <= m < m0 + nm:
                return i
        raise AssertionError

    with tile.TileContext(nc) as tc:
        with (
            tc.tile_pool(name="const", bufs=1) as constp,
            tc.tile_pool(name="gtp", bufs=10) as gtp,
            tc.tile_pool(name="vp", bufs=2) as vp,
            tc.tile_pool(name="sb", bufs=2) as sb,
            tc.tile_pool(name="ps_hop", bufs=1, space="PSUM") as ps_hop,
            tc.tile_pool(name="ps_tp", bufs=2, space="PSUM") as ps_tp,
            tc.tile_pool(name="ps_w", bufs=2, space="PSUM") as ps_w,
            tc.tile_pool(name="dram", bufs=2, space="DRAM") as dram,
        ):
            ident = constp.tile([P, P], f32)
            masks.make_identity(nc, ident[:])
            w_sb = constp.tile([F, ORDER * F], f32)
            nc.scalar.dma_start(w_sb[:], wp)
            xt_sb = constp.tile([F, rpc], f32)
            nc.scalar.dma_start(xt_sb[:], xt)
            out_sb = constp.tile([F, rpc], f32)
            pin = constp.tile([P, jc * 2 * pin_l], bf16)

            # v holds y_{k-1} as bf16 hi/lo pairs, one tile per fc part so
            # next-hop matmuls only depend on the partial gather that
            # produced their columns
            v_parts = []
            off = 0
            for i, w_ in enumerate(vcols):
                vt = vp.tile([P, w_], bf16, tag=f"v{i}", name=f"v{i}")
                nc.scalar.dma_start(vt[:], xthl[:, off : off + w_])
                off += w_
                v_parts.append(vt)

            def v_hi(vps, j):
                c, m = j // mc, j % mc
                i = part_of(m)
                m0, nm = parts[i]
                col = (c * 2 * nm + (m - m0)) * F
                return vps[i][:, col : col + F]

            def v_lo(vps, j):
                c, m = j // mc, j % mc
                i = part_of(m)
                m0, nm = parts[i]
                col = (c * 2 * nm + nm + (m - m0)) * F
                return vps[i][:, col : col + F]

            # k = 0 contribution: out^T = Wp_0^T @ x^T (pure fp32)
            for s, l in fchunks:
                pw = ps_w.tile([F, l], f32, tag="pw")
                nc.tensor.matmul(
                    pw[:], lhsT=w_sb[:, 0:F], rhs=xt_sb[:, s : s + l],
                    start=True, stop=True,
                )
                nc.vector.tensor_copy(out_sb[:, s : s + l], pw[:])

            # j-chunks are consumed in sweep (= gather-firing) order so
            # each hop starts on columns whose gather finished first; the
            # pinned sweep stays last: its gather is smallest and its
            # consumers come after a ~46us runway in the next hop
            sweep_order = list(range(nfc))
            part_rank = {i: r for r, i in enumerate(sweep_order)}
            jorder = sorted(range(jc), key=lambda j: (part_rank[part_of(j % mc)], j))

            def reload_v(i, cc_out, v_dst):
                # SWDGE (gpsimd) so the gather-gated reload can't convoy
                # the G stream on the shared HWDGE completion lanes; one
                # DMA per part (hi/lo interleaved per core block)
                nc.gpsimd.dma_start(
                    v_dst[i][:].rearrange("p (c m) -> p c m", c=ncores),
                    cc_out[:].rearrange("(c p) m -> p c m", p=P),
                )

            for k in range(1, ORDER):
                v_cur = v_parts
                if k < ORDER - 1:
                    v_next = [
                        vp.tile([P, w_], bf16, tag=f"v{i}", name=f"vn{i}")
                        for i, w_ in enumerate(vcols)
                    ]
                y_t = sb.tile([F, rpc], f32, tag="yT")
                js = jorder
                # hop: y_k^T = (G @ y_{k-1})^T via 3 bf16 hi/lo passes,
                # one sweep per fc chunk so partial all-gathers overlap
                # the remaining sweeps
                for i in sweep_order:
                    s, l = fchunks[i]
                    # when both hi and lo rhs fit one PSUM bank, fuse the
                    # two v_hi passes into a single 2l-column matmul and
                    # fold the halves with the epilogue DVE op instead
                    merged = 2 * l <= 512
                    hp = ps_hop.tile(
                        [F, 2 * l] if merged else [F, l],
                        f32, tag=f"hop{i}", name=f"hp{i}",
                    )
                    pinned = i == pin_i
                    for jn, j in enumerate(js):
                        if pinned:
                            g = pin[:, j * 2 * l : (j + 1) * 2 * l]
                            if k == 1:
                                nc.sync.dma_start(
                                    g, ghls[i][j * P : (j + 1) * P, :]
                                )
                        else:
                            gt = gtp.tile(
                                [P, 2 * l], bf16, tag=f"gt{i}", name="gt"
                            )
                            nc.sync.dma_start(
                                gt[:], ghls[i][j * P : (j + 1) * P, :]
                            )
                            g = gt[:]
                        gh = g[:, 0:l]
                        gl = g[:, l : 2 * l]
                        if merged:
                            nc.tensor.matmul(
                                hp[:], lhsT=v_hi(v_cur, j), rhs=g[:, 0 : 2 * l],
                                start=(jn == 0), stop=False,
                            )
                            nc.tensor.matmul(
                                hp[:, 0:l], lhsT=v_lo(v_cur, j), rhs=gh,
                                start=False, stop=(jn == jc - 1),
                            )
                        else:
                            for t, (lhs, rhs) in enumerate(
                                (
                                    (v_hi(v_cur, j), gh),
                                    (v_lo(v_cur, j), gh),
                                    (v_hi(v_cur, j), gl),
                                )
                            ):
                                nc.tensor.matmul(
                                    hp[:], lhsT=lhs, rhs=rhs,
                                    start=(jn == 0 and t == 0),
                                    stop=(jn == jc - 1 and t == 2),
                                )
                    # sweep epilogue: copy out (folding the merged
                    # halves), Wp contribution
                    if merged:
                        # walrus allows only one PSUM operand per DVE op
                        nc.vector.tensor_copy(y_t[:, s : s + l], hp[:, 0:l])
                        nc.vector.tensor_add(
                            y_t[:, s : s + l], y_t[:, s : s + l], hp[:, l : 2 * l]
                        )
                    else:
                        nc.vector.tensor_copy(y_t[:, s : s + l], hp[:])
                    pw = ps_w.tile([F, l], f32, tag="pw")
                    nc.tensor.matmul(
                        pw[:], lhsT=w_sb[:, k * F : (k + 1) * F],
                        rhs=y_t[:, s : s + l], start=True, stop=True,
                    )
                    nc.vector.tensor_add(
                        out_sb[:, s : s + l], out_sb[:, s : s + l], pw[:]
                    )
                    if k < ORDER - 1:
                        # transpose this sweep's rows to natural layout,
                        # split bf16 hi/lo, partial all-gather; the
                        # reload into the next hop's v happens there
                        m0, nm = parts[i]
                        stage = sb.tile(
                            [P, 2 * nm * F], bf16, tag=f"stage{i}",
                            name=f"stage{i}",
                        )
                        for mm in range(nm):
                            m = m0 + mm
                            tp = ps_tp.tile([P, F], f32, tag="tp", name="tp")
                            nc.tensor.transpose(
                                tp[:], y_t[:, m * P : (m + 1) * P],
                                ident[0:F, 0:F],
                            )
                            hi = stage[:, mm * F : (mm + 1) * F]
                            lo = stage[:, (nm + mm) * F : (nm + mm + 1) * F]
                            nc.vector.tensor_copy(hi, tp[:])
                            nc.vector.tensor_sub(lo, tp[:], hi)
                        cc_in = dram.tile(
                            [P, 2 * nm * F], bf16, tag=f"ccin{i}",
                            name=f"ccin{i}",
                        )
                        cc_out = dram.tile(
                            [ncores * P, 2 * nm * F], bf16, tag=f"ccout{i}",
                            name=f"ccout{i}",
                        )
                        nc.scalar.dma_start(cc_in[:], stage[:])
                        nc.gpsimd.collective_compute(
                            "AllGather",
                            mybir.AluOpType.bypass,
                            replica_groups=[list(range(ncores))],
                            ins=[cc_in.opt()],
                            outs=[cc_out.opt()],
                        )
                        reload_v(i, cc_out, v_next)
                if k < ORDER - 1:
                    v_parts = v_next

            nc.scalar.dma_start(out_t, out_sb[:])

    nc.compile()
    return nc


def get_nc(np_total=NP, ncores=NCORES):
    key = (np_total, ncores)
    if key not in _CACHE:
        _CACHE[key] = _build(np_total, ncores)
    return _CACHE[key]


def _bf16_pair(a):
    import ml_dtypes

    hi = a.astype(ml_dtypes.bfloat16)
    lo = (a - hi.astype(np.float32)).astype(ml_dtypes.bfloat16)
    return hi, lo


def prep_inputs(x, gso, weight, np_total=NP, ncores=NCORES):
    """Host-side shard prep. Returns in_maps for run_bass_kernel_spmd."""
    n = x.shape[0]
    rpc = np_total // ncores
    jc = np_total // P

    x = np.asarray(x, dtype=np.float32)
    gso = np.asarray(gso, dtype=np.float32)
    weight = np.asarray(weight, dtype=np.float32)

    wp = np.concatenate(
        [
            weight[0] - weight[2],
            weight[1] - 3.0 * weight[3],
            2.0 * weight[2],
            4.0 * weight[3],
        ],
        axis=1,
    ).astype(np.float32)  # [F, ORDER*F]

    xpad = np.zeros((np_total, F), dtype=np.float32)
    xpad[:n] = x
    gpad = np.zeros((np_total, np_total), dtype=np.float32)
    gpad[:n, :n] = gso
    g_hi, g_lo = _bf16_pair(gpad)

    # x as bf16 hi/lo pair in the per-part v layout:
    # for part (m0, nm): block col (c*nm + ml)*F + f = row (c*mc+m0+ml)*P + p
    x_hi, x_lo = _bf16_pair(xpad)
    mc = rpc // P
    parts = [(s // P, min(512, rpc - s) // P) for s in range(0, rpc, 512)]

    def part_x(m0, nm):
        # [P, (c, hi|lo, ml, f)] interleaved per core block
        hi = x_hi.reshape(ncores, mc, P, F)[:, m0 : m0 + nm].transpose(2, 0, 1, 3)
        lo = x_lo.reshape(ncores, mc, P, F)[:, m0 : m0 + nm].transpose(2, 0, 1, 3)
        return np.stack([hi, lo], axis=2).reshape(P, ncores * 2 * nm * F)

    xthl = np.ascontiguousarray(
        np.concatenate([part_x(m0, nm) for (m0, nm) in parts], axis=1)
    )

    fchunks = [(s, min(512, rpc - s)) for s in range(0, rpc, 512)]
    in_maps = []
    for c in range(ncores):
        rows = slice(c * rpc, (c + 1) * rpc)
        ght_c = g_hi[rows, :].T  # [np_total, rpc] bf16
        glt_c = g_lo[rows, :].T
        m = {"xthl": xthl, "wp": wp}
        m["xt"] = np.ascontiguousarray(xpad[rows, :].T)  # [F, rpc] fp32
        for i, (s, l) in enumerate(fchunks):
            # per-row [hi cols | lo cols] for this fc sweep
            m[f"ghl{i}"] = np.ascontiguousarray(
                np.concatenate(
                    [ght_c[:, s : s + l], glt_c[:, s : s + l]], axis=1
                )
            )
        in_maps.append(m)
    return in_maps


def assemble_output(results, n=N, ncores=NCORES):
    out_t = np.concatenate([results[c]["outT"] for c in range(ncores)], axis=1)
    return np.ascontiguousarray(out_t.T[:n]).astype(np.float32)


def kernel(x, gso, weight):
    import time

    from concourse import bass_utils

    nc = get_nc()
    in_maps = prep_inputs(x, gso, weight)
    last_err = None
    for attempt in range(3):
        try:
            res = bass_utils.run_bass_kernel_spmd(
                nc, in_maps, core_ids=list(range(NCORES))
            )
            return assemble_output(res.results)
        except Exception as e:  # transient device wedge: retry
            last_err = e
            time.sleep(5.0 * (attempt + 1))
    raise last_err



# revision 2
# speedup vs baseline: 1.5452x; 1.5452x over previous
"""ChebConv (order-4) GNN layer on 8 Trainium2 NeuronCores.

Reference computation (fp32):
    T0 = x, T1 = G x, Tk = 2 G T{k-1} - T{k-2}
    out = sum_k Tk @ W[k]          # [N, F] with N=10000, F=32
Rewritten in the power basis: y0 = x, yk = G y{k-1},
    out = sum_k yk @ Wp[k]  with
    Wp = [W0 - W2, W1 - 3 W3, 2 W2, 4 W3]   (exact modulo fp reassociation)

Strategy (v2, plain-bf16 + SBUF-pinned G):
  * G and the per-hop node features are cast to plain bf16 (fp32 PSUM
    accumulation). Measured rel-err ~4e-3 against the fp32 reference --
    comfortably inside the 2e-2 gate -- and it halves HBM bytes and
    triples PE throughput vs the previous hi/lo-split kernel.
  * Row-shard G over 8 cores (1280 padded cols of G^T each, pad
    10000 -> 10240). Each core holds G^T slice [10240, 1280] bf16
    (26 MB). 60 of the 80 128-row j-chunks (~20 MB) are pinned in SBUF
    during hop 1 and reused by hops 2-3, which then stream only 6.5 MB
    each -- hop 1 runs at the HBM roofline (~73 us), hops 2-3 at the
    PE roofline (~44 us).
  * Each hop computes y_k^T = (G y_{k-1})^T in 3 sweeps of <=512
    output columns (one PSUM bank per sweep): per 128-row j-chunk one
    bf16 matmul (lhsT = v[j] [128,32], rhs = G^T tile [128,<=512])
    accumulates over all 80 j-chunks. Pinned-chunk loads are spread
    across hop-1's sweeps (each sweep loads only its own columns) so
    hop-1 DMA stays balanced; streamed chunks are interleaved evenly
    between pinned ones so hops 2-3 never outrun the stream.
  * The Wp contraction happens on-chip from yk^T in full fp32:
    matmul(lhsT=Wp_k [32,32], rhs=ykT chunk), DVE-add into the
    transposed output accumulator; the k=0 term uses the host xT slice.
  * After each sweep (except in the last hop) its rows are PE-
    transposed ([32,128] -> [128,32] blocks), cast to bf16, and
    all-gathered in a partial collective (DRAM bounce) that overlaps
    the remaining sweeps. The reload into the next hop's per-part v
    tiles rides SWDGE (gpsimd) so the gather-gated DMA cannot convoy
    the G stream on the shared HWDGE completion lanes; j-chunks are
    consumed in gather-firing order so each hop starts on columns
    whose gather finished first.
  * Output is returned transposed ([32, 1280] per core); the host
    concatenates, transposes and drops padding.
"""

import sys

if "/opt/trn_rl_repo" not in sys.path:
    sys.path.insert(0, "/opt/trn_rl_repo")

import numpy as np

N = 10000
F = 32
ORDER = 4
NCORES = 8
P = 128
NP = 10240  # padded node count: divisible by NCORES * P
RPC = NP // NCORES  # rows per core (1280)
JC = NP // P  # global 128-row chunks (80)
MC = RPC // P  # local 128-row chunks per core (10)
STREAM_STRIDE = 4  # every 4th j-chunk is streamed, rest pinned in SBUF

_CACHE = {}


def _build(np_total, ncores, stream_stride):
    from concourse import bacc, masks, mybir, tile

    rpc = np_total // ncores
    jc = np_total // P
    mc = rpc // P
    f32 = mybir.dt.float32
    bf16 = mybir.dt.bfloat16
    fchunks = [(s, min(512, rpc - s)) for s in range(0, rpc, 512)]
    nfc = len(fchunks)
    # per-part m-chunk geometry: part i covers m-chunks [m0, m0+nm)
    parts = [(s // P, l // P) for s, l in fchunks]
    vcols = [ncores * nm * F for (m0, nm) in parts]

    nc = bacc.Bacc(
        "TRN2", target_bir_lowering=False, debug=False, num_devices=ncores
    )
    gt = nc.dram_tensor("gt", [np_total, rpc], bf16, kind="ExternalInput").ap()
    # x in per-part v layout: part (m0,nm) block col (c*nm + ml)*F + f
    #   = padded x row (c*mc + m0 + ml)*P + p
    xv = nc.dram_tensor("xv", [P, sum(vcols)], bf16, kind="ExternalInput").ap()
    xt = nc.dram_tensor("xt", [F, rpc], f32, kind="ExternalInput").ap()
    wp = nc.dram_tensor("wp", [F, ORDER * F], f32, kind="ExternalInput").ap()
    out_t = nc.dram_tensor("outT", [F, rpc], f32, kind="ExternalOutput").ap()

    def part_of(m):
        for i, (m0, nm) in enumerate(parts):
            if m0 <= m < m0 + nm:
                return i
        raise AssertionError

    # j-chunks grouped by gather part; within each part every
    # stream_stride-th is streamed from HBM each hop, the rest pinned
    part_js = [
        [j for j in range(jc) if part_of(j % mc) == i] for i in range(nfc)
    ]
    stream_js = set()
    for js in part_js:
        stream_js.update(js[stream_stride - 1 :: stream_stride])
    jorder = [j for js in part_js for j in js]

    with tile.TileContext(nc) as tc:
        with (
            tc.tile_pool(name="const", bufs=1) as constp,
            tc.tile_pool(name="gtp", bufs=4) as gtp,
            tc.tile_pool(name="vp", bufs=2) as vp,
            tc.tile_pool(name="sb", bufs=2) as sb,
            tc.tile_pool(name="ps_hop", bufs=1, space="PSUM") as ps_hop,
            tc.tile_pool(name="ps_tp", bufs=2, space="PSUM") as ps_tp,
            tc.tile_pool(name="ps_w", bufs=2, space="PSUM") as ps_w,
            tc.tile_pool(name="dram", bufs=2, space="DRAM") as dram,
        ):
            ident = constp.tile([P, P], f32)
            masks.make_identity(nc, ident[:])
            w_sb = constp.tile([F, ORDER * F], f32)
            nc.scalar.dma_start(w_sb[:], wp)
            xt_sb = constp.tile([F, rpc], f32)
            nc.scalar.dma_start(xt_sb[:], xt)
            out_sb = constp.tile([F, rpc], f32)
            pin = {
                j: constp.tile([P, rpc], bf16, name=f"pin{j}")
                for j in range(jc)
                if j not in stream_js
            }

            # v holds y_{k-1} as bf16, one tile per fc part so next-hop
            # matmuls only depend on the partial gather that produced
            # their columns
            v_parts = []
            off = 0
            for i, w_ in enumerate(vcols):
                vt = vp.tile([P, w_], bf16, tag=f"v{i}", name=f"v{i}")
                nc.scalar.dma_start(vt[:], xv[:, off : off + w_])
                off += w_
                v_parts.append(vt)

            def v_of(vps, j):
                c, m = j // mc, j % mc
                i = part_of(m)
                m0, nm = parts[i]
                col = (c * nm + (m - m0)) * F
                return vps[i][:, col : col + F]

            # k = 0 contribution: out^T = Wp_0^T @ x^T (pure fp32)
            for s, l in fchunks:
                pw = ps_w.tile([F, l], f32, tag="pw")
                nc.tensor.matmul(
                    pw[:], lhsT=w_sb[:, 0:F], rhs=xt_sb[:, s : s + l],
                    start=True, stop=True,
                )
                nc.vector.tensor_copy(out_sb[:, s : s + l], pw[:])

            for k in range(1, ORDER):
                v_cur = v_parts
                if k < ORDER - 1:
                    v_next = [
                        vp.tile([P, w_], bf16, tag=f"v{i}", name=f"vn{i}")
                        for i, w_ in enumerate(vcols)
                    ]
                y_t = sb.tile([F, rpc], f32, tag="yT")
                # hop: y_k^T = (G @ y_{k-1})^T, one sweep per fc chunk
                # so partial all-gathers overlap the remaining sweeps
                for i, (s, l) in enumerate(fchunks):
                    hp = ps_hop.tile([F, l], f32, tag=f"hop{i}", name=f"hp{i}")
                    for jn, j in enumerate(jorder):
                        if j in pin:
                            g = pin[j][:, s : s + l]
                            if k == 1:
                                nc.sync.dma_start(
                                    g, gt[j * P : (j + 1) * P, s : s + l]
                                )
                        else:
                            t = gtp.tile([P, l], bf16, tag=f"gt{i}", name="gt")
                            nc.sync.dma_start(
                                t[:], gt[j * P : (j + 1) * P, s : s + l]
                            )
                            g = t[:]
                        nc.tensor.matmul(
                            hp[:], lhsT=v_of(v_cur, j), rhs=g,
                            start=(jn == 0), stop=(jn == jc - 1),
                        )
                    # sweep epilogue: PSUM -> SBUF, Wp contribution
                    nc.vector.tensor_copy(y_t[:, s : s + l], hp[:])
                    pw = ps_w.tile([F, l], f32, tag="pw")
                    nc.tensor.matmul(
                        pw[:], lhsT=w_sb[:, k * F : (k + 1) * F],
                        rhs=y_t[:, s : s + l], start=True, stop=True,
                    )
                    nc.vector.tensor_add(
                        out_sb[:, s : s + l], out_sb[:, s : s + l], pw[:]
                    )
                    if k < ORDER - 1:
                        # transpose this sweep's rows to natural layout,
                        # cast bf16, partial all-gather; the reload into
                        # the next hop's v happens on SWDGE
                        m0, nm = parts[i]
                        stage = sb.tile(
                            [P, nm * F], bf16, tag=f"stage{i}",
                            name=f"stage{i}",
                        )
                        for mm in range(nm):
                            m = m0 + mm
                            tp = ps_tp.tile([P, F], f32, tag="tp", name="tp")
                            nc.tensor.transpose(
                                tp[:], y_t[:, m * P : (m + 1) * P],
                                ident[0:F, 0:F],
                            )
                            nc.vector.tensor_copy(
                                stage[:, mm * F : (mm + 1) * F], tp[:]
                            )
                        cc_in = dram.tile(
                            [P, nm * F], bf16, tag=f"ccin{i}", name=f"ccin{i}"
                        )
                        cc_out = dram.tile(
                            [ncores * P, nm * F], bf16, tag=f"ccout{i}",
                            name=f"ccout{i}",
                        )
                        nc.scalar.dma_start(cc_in[:], stage[:])
                        nc.gpsimd.collective_compute(
                            "AllGather",
                            mybir.AluOpType.bypass,
                            replica_groups=[list(range(ncores))],
                            ins=[cc_in.opt()],
                            outs=[cc_out.opt()],
                        )
                        nc.gpsimd.dma_start(
                            v_next[i][:].rearrange("p (c m) -> p c m", c=ncores),
                            cc_out[:].rearrange("(c p) m -> p c m", p=P),
                        )
                if k < ORDER - 1:
                    v_parts = v_next

            nc.scalar.dma_start(out_t, out_sb[:])

    nc.compile()
    return nc


def get_nc(np_total=NP, ncores=NCORES, stream_stride=STREAM_STRIDE):
    key = (np_total, ncores, stream_stride)
    if key not in _CACHE:
        _CACHE[key] = _build(np_total, ncores, stream_stride)
    return _CACHE[key]


def prep_inputs(x, gso, weight, np_total=NP, ncores=NCORES):
    """Host-side shard prep. Returns in_maps for run_bass_kernel_spmd."""
    import ml_dtypes

    bf = ml_dtypes.bfloat16
    n = x.shape[0]
    rpc = np_total // ncores
    mc = rpc // P

    x = np.asarray(x, dtype=np.float32)
    gso = np.asarray(gso, dtype=np.float32)
    weight = np.asarray(weight, dtype=np.float32)

    wp = np.concatenate(
        [
            weight[0] - weight[2],
            weight[1] - 3.0 * weight[3],
            2.0 * weight[2],
            4.0 * weight[3],
        ],
        axis=1,
    ).astype(np.float32)  # [F, ORDER*F]

    xpad = np.zeros((np_total, F), dtype=np.float32)
    xpad[:n] = x
    gpad = np.zeros((np_total, np_total), dtype=np.float32)
    gpad[:n, :n] = gso
    g16 = gpad.astype(bf)

    # x as bf16 in the per-part v layout:
    # part (m0, nm): block col (c*nm + ml)*F + f = row (c*mc+m0+ml)*P + p
    x16 = xpad.astype(bf)
    parts = [(s // P, min(512, rpc - s) // P) for s in range(0, rpc, 512)]

    def part_x(m0, nm):
        return (
            x16.reshape(ncores, mc, P, F)[:, m0 : m0 + nm]
            .transpose(2, 0, 1, 3)
            .reshape(P, ncores * nm * F)
        )

    xv = np.ascontiguousarray(
        np.concatenate([part_x(m0, nm) for (m0, nm) in parts], axis=1)
    )

    in_maps = []
    for c in range(ncores):
        rows = slice(c * rpc, (c + 1) * rpc)
        in_maps.append(
            {
                "gt": np.ascontiguousarray(g16[rows, :].T),  # [np_total, rpc]
                "xv": xv,
                "wp": wp,
                "xt": np.ascontiguousarray(xpad[rows, :].T),  # [F, rpc] fp32
            }
        )
    return in_maps


def assemble_output(results, n=N, ncores=NCORES):
    out_t = np.concatenate([results[c]["outT"] for c in range(ncores)], axis=1)
    return np.ascontiguousarray(out_t.T[:n]).astype(np.float32)


def kernel(x, gso, weight):
    import time

    from concourse import bass_utils

    nc = get_nc()
    in_maps = prep_inputs(x, gso, weight)
    last_err = None
    for attempt in range(3):
        try:
            res = bass_utils.run_bass_kernel_spmd(
                nc, in_maps, core_ids=list(range(NCORES))
            )
            return assemble_output(res.results)
        except Exception as e:  # transient device wedge: retry
            last_err = e
            time.sleep(5.0 * (attempt + 1))
    raise last_err


# revision 4
# speedup vs baseline: 1.9307x; 1.2494x over previous
"""ChebConv (order-4) GNN layer on 8 Trainium2 NeuronCores.

Reference computation (fp32):
    T0 = x, T1 = G x, Tk = 2 G T{k-1} - T{k-2}
    out = sum_k Tk @ W[k]          # [N, F] with N=10000, F=32
Rewritten in the power basis: y0 = x, yk = G y{k-1},
    out = sum_k yk @ Wp[k]  with
    Wp = [W0 - W2, W1 - 3 W3, 2 W2, 4 W3]   (exact modulo fp reassociation)

Strategy (v3, plain-bf16 + SBUF-pinned G + batched DMA):
  * G, the per-hop node features, and Wp[1:] are plain bf16 with fp32
    PSUM accumulation (rel-err ~4e-3 vs the 2e-2 gate); this halves HBM
    bytes and triples PE throughput vs the original hi/lo-split kernel.
  * Row-shard G over 8 cores (1280 padded cols of G^T each, pad
    10000 -> 10240). Per core, 56 of the 80 128-row j-chunks of the
    G^T slice (~18 MB bf16) are pinned in SBUF during hop 1 and reused
    by hops 2-3, which then stream only ~6.5 MB each: hop 1 runs at
    the HBM roofline (~75 us), hops 2-3 at the PE roofline (~45 us).
  * DMA is batched to sidestep the ~0.6 us/trigger HWDGE issue cost
    that serialized a previous per-j-chunk version (360 triggers ->
    ~90): the host pre-groups pinned chunks per (core-block, gather
    part) into contiguous runs loaded by one 3-level-AP DMA each (in
    two column chunks so hop-1 sweeps stay DMA-balanced), and the 8
    streamed chunks of a gather part load as one strided batch per
    sweep into a 4-deep uniform ring. Pin loads ride the sync queue,
    everything else the scalar queue, so ring-reuse waits can never
    head-of-line-block the G stream.
  * Each hop computes y_k^T in 3 sweeps of <=512 output columns (one
    PSUM bank per sweep): per j-chunk one bf16 matmul (lhsT = v[j]
    [128,32], rhs = G^T tile [128,<=512]) accumulates over all 80
    chunks. The sweep epilogue copies PSUM to a bf16 y16 row, adds the
    bf16 Wp_k contribution to the fp32 output accumulator (bf16 rhs
    streams at full PE rate; fp32 would run LOW_HIGH at half rate),
    PE-transposes the sweep's rows, and fires a partial AllGather that
    overlaps the remaining sweeps; the reload into the next hop's
    per-part v tiles rides SWDGE (gpsimd). j-chunks are consumed in
    gather-firing order so each hop starts on columns whose gather
    finished first. The k=0 term uses the host fp32 xT slice.
  * A dummy 8 KB AllGather is issued at kernel start: the collective
    runtime runs a ~50-85 us one-time init barrier on the CC cores
    (starting ~21 us into the NEFF), and without the dummy the first
    real gather - which gates hop 2 - queues behind it.
  * Output is returned transposed ([32, 1280] per core); the host
    concatenates, transposes and drops padding.
"""

import sys

if "/opt/trn_rl_repo" not in sys.path:
    sys.path.insert(0, "/opt/trn_rl_repo")

import numpy as np

N = 10000
F = 32
ORDER = 4
NCORES = 8
P = 128
NP = 10240  # padded node count: divisible by NCORES * P
RPC = NP // NCORES  # rows per core (1280)
JC = NP // P  # global 128-row chunks (80)
MC = RPC // P  # local 128-row chunks per core (10)

# gather parts (m-chunk ranges per fc sweep) and the m-chunks of each
# part that are pinned in SBUF vs streamed every hop
PARTS = [(0, 4), (4, 4), (8, 2)]
PIN_MS = [[0, 1, 2], [4, 5, 6], [8]]
STR_MS = [3, 7, 9]
NPIN = sum(len(ms) for ms in PIN_MS)  # pinned m-chunks per core-block (7)
PIN_SPLIT = 512  # pin loads split at this column for hop-1 DMA balance

_CACHE = {}


def _build(np_total, ncores):
    from concourse import bacc, masks, mybir, tile

    rpc = np_total // ncores
    jc = np_total // P
    mc = rpc // P
    f32 = mybir.dt.float32
    bf16 = mybir.dt.bfloat16
    fchunks = [(s, min(512, rpc - s)) for s in range(0, rpc, 512)]
    nfc = len(fchunks)
    vcols = [ncores * nm * F for (m0, nm) in PARTS]
    n_str = len(STR_MS) * ncores  # streamed j-chunks (24)

    nc = bacc.Bacc(
        "TRN2", target_bir_lowering=False, debug=False, num_devices=ncores
    )
    # pinned chunks, rows grouped (c, part, m-in-run, p)
    g_pin = nc.dram_tensor(
        "g_pin", [ncores * NPIN * P, rpc], bf16, kind="ExternalInput"
    ).ap()
    # streamed chunks, rows grouped (part, c, p)
    g_str = nc.dram_tensor(
        "g_str", [n_str * P, rpc], bf16, kind="ExternalInput"
    ).ap()
    # x in per-part v layout: part (m0,nm) block col (c*nm + ml)*F + f
    #   = padded x row (c*mc + m0 + ml)*P + p
    xv = nc.dram_tensor("xv", [P, sum(vcols)], bf16, kind="ExternalInput").ap()
    xt = nc.dram_tensor("xt", [F, rpc], f32, kind="ExternalInput").ap()
    wp = nc.dram_tensor("wp", [F, ORDER * F], f32, kind="ExternalInput").ap()
    out_t = nc.dram_tensor("outT", [F, rpc], f32, kind="ExternalOutput").ap()

    def part_of(m):
        for i, (m0, nm) in enumerate(PARTS):
            if m0 <= m < m0 + nm:
                return i
        raise AssertionError

    # pinned-run geometry: run (c, part p) starts at this g_pin row
    pin_row0 = {}
    r = 0
    for c in range(ncores):
        for pi in range(nfc):
            pin_row0[(c, pi)] = r
            r += len(PIN_MS[pi]) * P

    # j consumption order: parts in gather-firing order; within a part
    # all pinned chunks (c-major, matching pin-load arrival), then the
    # streamed batch (whose load prefetches at sweep start)
    jorder = []
    for pi in range(nfc):
        for c in range(ncores):
            jorder += [c * mc + m for m in PIN_MS[pi]]
        jorder += [c * mc + STR_MS[pi] for c in range(ncores)]

    lo_w = PIN_SPLIT
    hi_w = rpc - PIN_SPLIT

    with tile.TileContext(nc) as tc:
        with (
            tc.tile_pool(name="const", bufs=1) as constp,
            tc.tile_pool(name="gsp", bufs=4) as gsp,
            tc.tile_pool(name="vp", bufs=2) as vp,
            tc.tile_pool(name="sb", bufs=2) as sb,
            tc.tile_pool(name="ps_hop", bufs=1, space="PSUM") as ps_hop,
            tc.tile_pool(name="ps_tp", bufs=2, space="PSUM") as ps_tp,
            tc.tile_pool(name="ps_w", bufs=2, space="PSUM") as ps_w,
            tc.tile_pool(name="dram", bufs=2, space="DRAM") as dram,
        ):
            ident = constp.tile([P, P], f32)
            masks.make_identity(nc, ident[:])
            w_sb = constp.tile([F, ORDER * F], f32)
            nc.scalar.dma_start(w_sb[:], wp)
            xt_sb = constp.tile([F, rpc], f32)
            nc.scalar.dma_start(xt_sb[:], xt)
            out_sb = constp.tile([F, rpc], f32)
            ident16 = constp.tile([F, F], bf16)
            nc.vector.tensor_copy(ident16[:], ident[0:F, 0:F])
            w16 = constp.tile([F, ORDER * F], bf16)
            nc.vector.tensor_copy(w16[:], w_sb[:])

            # dummy collective to absorb the CC-core init barrier
            d_sb = constp.tile([P, F], bf16)
            nc.vector.tensor_copy(d_sb[:], ident[:, 0:F])
            d_in = dram.tile([P, F], bf16, tag="d_in", name="d_in")
            d_out = dram.tile([ncores * P, F], bf16, tag="d_out", name="d_out")
            nc.scalar.dma_start(d_in[:], d_sb[:])
            nc.gpsimd.collective_compute(
                "AllGather",
                mybir.AluOpType.bypass,
                replica_groups=[list(range(ncores))],
                ins=[d_in.opt()],
                outs=[d_out.opt()],
            )

            # pinned G: two tiles per (core-block, part) - cols
            # [0:PIN_SPLIT) and [PIN_SPLIT:rpc) of each chunk in the run
            pin_lo, pin_hi = {}, {}
            for c in range(ncores):
                for pi in range(nfc):
                    npin = len(PIN_MS[pi])
                    pin_lo[(c, pi)] = constp.tile(
                        [P, npin * lo_w], bf16, name=f"plo{c}_{pi}"
                    )
                    pin_hi[(c, pi)] = constp.tile(
                        [P, npin * hi_w], bf16, name=f"phi{c}_{pi}"
                    )

            def pin_load(dst, col0, col1, c, pi):
                npin = len(PIN_MS[pi])
                r0 = pin_row0[(c, pi)]
                nc.sync.dma_start(
                    dst[:].rearrange("p (a f) -> p a f", a=npin),
                    g_pin[r0 : r0 + npin * P, col0:col1].rearrange(
                        "(a p) f -> p a f", p=P
                    ),
                )

            def pin_rhs(j, s, l):
                c, m = j // mc, j % mc
                pi = part_of(m)
                a = PIN_MS[pi].index(m)
                if s + l <= PIN_SPLIT:
                    t = pin_lo[(c, pi)]
                    col = a * lo_w + s
                else:
                    t = pin_hi[(c, pi)]
                    col = a * hi_w + (s - PIN_SPLIT)
                return t[:, col : col + l]

            # v holds y_{k-1} as bf16, one tile per part so next-hop
            # matmuls only depend on the partial gather that produced
            # their columns
            v_parts = []
            off = 0
            for i, w_ in enumerate(vcols):
                vt = vp.tile([P, w_], bf16, tag=f"v{i}", name=f"v{i}")
                nc.scalar.dma_start(vt[:], xv[:, off : off + w_])
                off += w_
                v_parts.append(vt)

            def v_of(vps, j):
                c, m = j // mc, j % mc
                i = part_of(m)
                m0, nm = PARTS[i]
                col = (c * nm + (m - m0)) * F
                return vps[i][:, col : col + F]

            # k = 0 contribution: out^T = Wp_0^T @ x^T (pure fp32)
            for s, l in fchunks:
                pw = ps_w.tile([F, l], f32, tag="pw")
                nc.tensor.matmul(
                    pw[:], lhsT=w_sb[:, 0:F], rhs=xt_sb[:, s : s + l],
                    start=True, stop=True,
                )
                nc.vector.tensor_copy(out_sb[:, s : s + l], pw[:])

            for k in range(1, ORDER):
                v_cur = v_parts
                if k < ORDER - 1:
                    v_next = [
                        vp.tile([P, w_], bf16, tag=f"v{i}", name=f"vn{i}")
                        for i, w_ in enumerate(vcols)
                    ]
                y16 = sb.tile([F, rpc], bf16, tag="y16")
                # hop: y_k^T = (G @ y_{k-1})^T, one sweep per fc chunk
                # so partial all-gathers overlap the remaining sweeps
                for i, (s, l) in enumerate(fchunks):
                    # hop-1 pin loads: lo cols during sweep 0, hi during
                    # sweep 1 (balances hop-1 DMA across sweeps)
                    if k == 1 and i < 2:
                        for pi in range(nfc):
                            for c in range(ncores):
                                if i == 0:
                                    pin_load(
                                        pin_lo[(c, pi)], 0, PIN_SPLIT, c, pi
                                    )
                                else:
                                    pin_load(
                                        pin_hi[(c, pi)], PIN_SPLIT, rpc, c, pi
                                    )
                    # streamed chunks: one strided batch per part into a
                    # uniform 4-deep ring (sweep 2 uses half the tile)
                    sbt = {}
                    for pi in range(nfc):
                        t = gsp.tile([P, ncores * 512], bf16, tag="gs", name="gs")
                        nc.sync.dma_start(
                            t[:, 0 : ncores * l].rearrange(
                                "p (c f) -> p c f", c=ncores
                            ),
                            g_str[
                                pi * ncores * P : (pi + 1) * ncores * P,
                                s : s + l,
                            ].rearrange("(c p) f -> p c f", p=P),
                        )
                        sbt[pi] = t
                    hp = ps_hop.tile([F, l], f32, tag=f"hop{i}", name=f"hp{i}")
                    for jn, j in enumerate(jorder):
                        c, m = j // mc, j % mc
                        if m in STR_MS:
                            pi = part_of(m)
                            g = sbt[pi][:, c * l : (c + 1) * l]
                        else:
                            g = pin_rhs(j, s, l)
                        nc.tensor.matmul(
                            hp[:], lhsT=v_of(v_cur, j), rhs=g,
                            start=(jn == 0), stop=(jn == jc - 1),
                        )
                    # sweep epilogue: PSUM -> bf16 y16, Wp contribution
                    nc.vector.tensor_copy(y16[:, s : s + l], hp[:])
                    pw = ps_w.tile([F, l], f32, tag="pw")
                    nc.tensor.matmul(
                        pw[:], lhsT=w16[:, k * F : (k + 1) * F],
                        rhs=y16[:, s : s + l], start=True, stop=True,
                    )
                    nc.vector.tensor_add(
                        out_sb[:, s : s + l], out_sb[:, s : s + l], pw[:]
                    )
                    if k < ORDER - 1:
                        # transpose this sweep's rows to natural layout,
                        # partial all-gather; the reload into the next
                        # hop's v rides SWDGE
                        m0, nm = PARTS[i]
                        stage = sb.tile(
                            [P, nm * F], bf16, tag=f"stage{i}",
                            name=f"stage{i}",
                        )
                        for mm in range(nm):
                            m = m0 + mm
                            tp = ps_tp.tile([P, F], bf16, tag="tp", name="tp")
                            nc.tensor.transpose(
                                tp[:], y16[:, m * P : (m + 1) * P],
                                ident16[:],
                            )
                            nc.vector.tensor_copy(
                                stage[:, mm * F : (mm + 1) * F], tp[:]
                            )
                        cc_in = dram.tile(
                            [P, nm * F], bf16, tag=f"ccin{i}", name=f"ccin{i}"
                        )
                        cc_out = dram.tile(
                            [ncores * P, nm * F], bf16, tag=f"ccout{i}",
                            name=f"ccout{i}",
                        )
                        nc.scalar.dma_start(cc_in[:], stage[:])
                        nc.gpsimd.collective_compute(
                            "AllGather",
                            mybir.AluOpType.bypass,
                            replica_groups=[list(range(ncores))],
                            ins=[cc_in.opt()],
                            outs=[cc_out.opt()],
                        )
                        nc.gpsimd.dma_start(
                            v_next[i][:].rearrange(
                                "p (c m) -> p c m", c=ncores
                            ),
                            cc_out[:].rearrange("(c p) m -> p c m", p=P),
                        )
                if k < ORDER - 1:
                    v_parts = v_next

            nc.scalar.dma_start(out_t, out_sb[:])

    nc.compile()
    return nc


def get_nc(np_total=NP, ncores=NCORES):
    key = (np_total, ncores)
    if key not in _CACHE:
        _CACHE[key] = _build(np_total, ncores)
    return _CACHE[key]


def prep_inputs(x, gso, weight, np_total=NP, ncores=NCORES):
    """Host-side shard prep. Returns in_maps for run_bass_kernel_spmd."""
    import ml_dtypes

    bf = ml_dtypes.bfloat16
    n = x.shape[0]
    rpc = np_total // ncores
    mc = rpc // P

    x = np.asarray(x, dtype=np.float32)
    gso = np.asarray(gso, dtype=np.float32)
    weight = np.asarray(weight, dtype=np.float32)

    wp = np.concatenate(
        [
            weight[0] - weight[2],
            weight[1] - 3.0 * weight[3],
            2.0 * weight[2],
            4.0 * weight[3],
        ],
        axis=1,
    ).astype(np.float32)  # [F, ORDER*F]

    xpad = np.zeros((np_total, F), dtype=np.float32)
    xpad[:n] = x
    gpad = np.zeros((np_total, np_total), dtype=np.float32)
    gpad[:n, :n] = gso
    g16 = gpad.astype(bf)

    # x as bf16 in the per-part v layout
    x16 = xpad.astype(bf)

    def part_x(m0, nm):
        return (
            x16.reshape(ncores, mc, P, F)[:, m0 : m0 + nm]
            .transpose(2, 0, 1, 3)
            .reshape(P, ncores * nm * F)
        )

    xv = np.ascontiguousarray(
        np.concatenate([part_x(m0, nm) for (m0, nm) in PARTS], axis=1)
    )

    in_maps = []
    for c in range(ncores):
        rows = slice(c * rpc, (c + 1) * rpc)
        gt = np.ascontiguousarray(g16[rows, :].T)  # [np_total, rpc]
        gt4 = gt.reshape(ncores, mc, P, rpc)
        pin_rows = np.concatenate(
            [
                gt4[cb, m]
                for cb in range(ncores)
                for ms in PIN_MS
                for m in ms
            ],
            axis=0,
        )
        str_rows = np.concatenate(
            [gt4[cb, m] for m in STR_MS for cb in range(ncores)], axis=0
        )
        in_maps.append(
            {
                "g_pin": np.ascontiguousarray(pin_rows),
                "g_str": np.ascontiguousarray(str_rows),
                "xv": xv,
                "wp": wp,
                "xt": np.ascontiguousarray(xpad[rows, :].T),  # [F, rpc] f32
            }
        )
    return in_maps


def assemble_output(results, n=N, ncores=NCORES):
    out_t = np.concatenate([results[c]["outT"] for c in range(ncores)], axis=1)
    return np.ascontiguousarray(out_t.T[:n]).astype(np.float32)


def kernel(x, gso, weight):
    import time

    from concourse import bass_utils

    nc = get_nc()
    in_maps = prep_inputs(x, gso, weight)
    last_err = None
    for attempt in range(3):
        try:
            res = bass_utils.run_bass_kernel_spmd(
                nc, in_maps, core_ids=list(range(NCORES))
            )
            return assemble_output(res.results)
        except Exception as e:  # transient device wedge: retry
            last_err = e
            time.sleep(5.0 * (attempt + 1))
    raise last_err


# revision 5
# speedup vs baseline: 2.0126x; 1.0424x over previous
"""ChebConv (order-4) GNN layer on 8 Trainium2 NeuronCores.

Reference computation (fp32):
    T0 = x, T1 = G x, Tk = 2 G T{k-1} - T{k-2}
    out = sum_k Tk @ W[k]          # [N, F] with N=10000, F=32
Rewritten in the power basis: y0 = x, yk = G y{k-1},
    out = sum_k yk @ Wp[k]  with
    Wp = [W0 - W2, W1 - 3 W3, 2 W2, 4 W3]   (exact modulo fp reassociation)

Strategy (v4, plain-bf16 + SBUF-pinned G + coarse DMA + gather-first
sweep order):
  * G, the per-hop node features, and Wp[1:] are plain bf16 with fp32
    PSUM accumulation (rel-err ~4e-3 vs the 2e-2 gate); this halves HBM
    bytes and triples PE throughput vs the original hi/lo-split kernel.
  * Row-shard G over 8 cores (1280 padded cols of G^T each, pad
    10000 -> 10240). Per core, 56 of the 80 128-row j-chunks of the
    G^T slice (~18 MB bf16) are pinned in SBUF during hop 1 and reused
    by hops 2-3, which then stream only ~6.5 MB each: hop 1 runs at
    the HBM roofline (~75 us), hops 2-3 at the PE roofline (~45 us).
  * HWDGE trigger instructions cost ~0.6-1.3 us EACH on the issuing
    engine queue, so DMA must be coarse: the host pre-groups G^T rows
    so each (gather-part, sweep-column-chunk) of the pinned set loads
    with ONE 3-level-AP DMA (9 pin triggers total, issued per sweep of
    hop 1 in consumption order) and the 8 streamed chunks of a part
    load as one strided batch per sweep (9/hop) into a 4-deep ring.
  * Sweeps run in gather-part order [2-chunk part, 4, 4] (the host
    permutes G^T/x^T columns so sweep columns stay contiguous): the
    smallest part's all-gather fires first and each hop consumes
    j-chunks in the same part order, so hop k+1 can start as soon as
    the first (smallest, earliest-fired) gather of hop k lands while
    later parts' gathers complete behind it.  Collectives have a
    ~5-15 us floor and the CC runtime runs a ~50-85 us one-time init
    barrier (starting ~21 us into the NEFF) which gates the first
    gather - the sweep order keeps everything after that floor
    pipelined.  Reloads of gathered y into the next hop's per-part v
    tiles ride the scalar queue so the CC queue runs gathers
    back-to-back.
  * Each hop computes y_k^T in 3 sweeps (one PSUM bank per sweep): per
    j-chunk one bf16 matmul (lhsT = v[j] [128,32], rhs = G^T tile
    [128,<=512]) accumulates over all 80 chunks; the epilogue copies
    PSUM to bf16 y16, adds the bf16 Wp_k term into the fp32 output
    accumulator (bf16 rhs streams at full PE rate; fp32 runs LOW_HIGH
    at half rate), PE-transposes the sweep rows and fires the partial
    AllGather. The k=0 term uses the host fp32 xT slice.
  * Output is returned transposed and column-permuted ([32, 1280] per
    core); the host concatenates, un-permutes, transposes and drops
    padding.
"""

import sys

if "/opt/trn_rl_repo" not in sys.path:
    sys.path.insert(0, "/opt/trn_rl_repo")

import numpy as np

N = 10000
F = 32
ORDER = 4
NCORES = 8
P = 128
NP = 10240  # padded node count: divisible by NCORES * P
RPC = NP // NCORES  # rows per core (1280)
JC = NP // P  # global 128-row chunks (80)
MC = RPC // P  # local 128-row chunks per core (10)

# gather parts in sweep order; per part: natural m-chunks (host permutes
# columns to this order), pinned m-chunks, streamed m-chunk
PART_MS = [[8, 9], [0, 1, 2, 3], [4, 5, 6, 7]]
PIN_MS = [[8], [0, 1, 2], [4, 5, 6]]
STR_MS = [9, 3, 7]
NEW_MS = [m for ms in PART_MS for m in ms]  # host column permutation

_CACHE = {}


def _build(np_total, ncores):
    from concourse import bacc, masks, mybir, tile

    rpc = np_total // ncores
    jc = np_total // P
    mc = rpc // P
    f32 = mybir.dt.float32
    bf16 = mybir.dt.bfloat16
    nfc = len(PART_MS)
    # sweep column ranges (permuted space) == part ranges
    parts = []
    s = 0
    for ms in PART_MS:
        parts.append((s // P, len(ms)))
        s += len(ms) * P
    fchunks = [(m0 * P, nm * P) for (m0, nm) in parts]
    vcols = [ncores * nm * F for (m0, nm) in parts]
    n_pin = sum(len(ms) for ms in PIN_MS) * ncores  # 56
    n_str = len(STR_MS) * ncores  # 24

    nc = bacc.Bacc(
        "TRN2", target_bir_lowering=False, debug=False, num_devices=ncores
    )
    # pinned chunks, rows grouped (part, c, m-in-run, p)
    g_pin = nc.dram_tensor(
        "g_pin", [n_pin * P, rpc], bf16, kind="ExternalInput"
    ).ap()
    # streamed chunks, rows grouped (part, c, p)
    g_str = nc.dram_tensor(
        "g_str", [n_str * P, rpc], bf16, kind="ExternalInput"
    ).ap()
    # x in per-part v layout: part block col (c*nm + ml)*F + f
    #   = padded x row (c*mc + PART_MS[part][ml])*P + p
    xv = nc.dram_tensor("xv", [P, sum(vcols)], bf16, kind="ExternalInput").ap()
    xt = nc.dram_tensor("xt", [F, rpc], f32, kind="ExternalInput").ap()
    wp = nc.dram_tensor("wp", [F, ORDER * F], f32, kind="ExternalInput").ap()
    out_t = nc.dram_tensor("outT", [F, rpc], f32, kind="ExternalOutput").ap()

    # g_pin row offset of each part block (units of P rows)
    pin_part0 = []
    r = 0
    for pi in range(nfc):
        pin_part0.append(r)
        r += ncores * len(PIN_MS[pi])

    # j lookup tables (natural m space)
    m2part = {}
    for pi, ms in enumerate(PART_MS):
        for ml, m in enumerate(ms):
            m2part[m] = (pi, ml)

    # consumption order: parts in gather-firing order; within a part
    # pinned chunks (c-major, matching the one-DMA pin-tile arrival),
    # then the streamed batch
    jorder = []
    for pi in range(nfc):
        jorder += [c * mc + m for c in range(ncores) for m in PIN_MS[pi]]
        jorder += [c * mc + STR_MS[pi] for c in range(ncores)]

    with tile.TileContext(nc) as tc:
        with (
            tc.tile_pool(name="const", bufs=1) as constp,
            tc.tile_pool(name="gsp", bufs=4) as gsp,
            tc.tile_pool(name="vp", bufs=2) as vp,
            tc.tile_pool(name="sb", bufs=2) as sb,
            tc.tile_pool(name="ps_hop", bufs=1, space="PSUM") as ps_hop,
            tc.tile_pool(name="ps_tp", bufs=2, space="PSUM") as ps_tp,
            tc.tile_pool(name="ps_w", bufs=2, space="PSUM") as ps_w,
            tc.tile_pool(name="dram", bufs=2, space="DRAM") as dram,
        ):
            ident = constp.tile([P, P], f32)
            masks.make_identity(nc, ident[:])
            xt_sb = constp.tile([F, rpc], f32)
            nc.scalar.dma_start(xt_sb[:], xt)
            w_sb = constp.tile([F, ORDER * F], f32)
            nc.scalar.dma_start(w_sb[:], wp)
            out_sb = constp.tile([F, rpc], f32)
            ident16 = constp.tile([F, F], bf16)
            nc.vector.tensor_copy(ident16[:], ident[0:F, 0:F])
            w16 = constp.tile([F, ORDER * F], bf16)
            nc.vector.tensor_copy(w16[:], w_sb[:])

            # pinned G: one tile per (part, sweep col-chunk), loaded by a
            # single 3-level-AP DMA during that sweep of hop 1
            pin = {}
            for pi in range(nfc):
                na = ncores * len(PIN_MS[pi])
                for i, (s, l) in enumerate(fchunks):
                    pin[(pi, i)] = constp.tile(
                        [P, na * l], bf16, name=f"pin{pi}_{i}"
                    )

            def pin_load(pi, i):
                na = ncores * len(PIN_MS[pi])
                s, l = fchunks[i]
                r0 = pin_part0[pi] * P
                nc.sync.dma_start(
                    pin[(pi, i)][:].rearrange("p (a f) -> p a f", a=na),
                    g_pin[r0 : r0 + na * P, s : s + l].rearrange(
                        "(a p) f -> p a f", p=P
                    ),
                )

            # v holds y_{k-1} as bf16, one tile per part so next-hop
            # matmuls only depend on the partial gather that produced
            # their columns
            v_parts = []
            off = 0
            for i, w_ in enumerate(vcols):
                vt = vp.tile([P, w_], bf16, tag=f"v{i}", name=f"v{i}")
                nc.scalar.dma_start(vt[:], xv[:, off : off + w_])
                off += w_
                v_parts.append(vt)

            def v_of(vps, j):
                c, m = j // mc, j % mc
                pi, ml = m2part[m]
                nm = len(PART_MS[pi])
                col = (c * nm + ml) * F
                return vps[pi][:, col : col + F]

            # k = 0 contribution: out^T = Wp_0^T @ x^T (pure fp32)
            for s, l in fchunks:
                pw = ps_w.tile([F, l], f32, tag="pw")
                nc.tensor.matmul(
                    pw[:], lhsT=w_sb[:, 0:F], rhs=xt_sb[:, s : s + l],
                    start=True, stop=True,
                )
                nc.vector.tensor_copy(out_sb[:, s : s + l], pw[:])

            for k in range(1, ORDER):
                v_cur = v_parts
                if k < ORDER - 1:
                    v_next = [
                        vp.tile([P, w_], bf16, tag=f"v{i}", name=f"vn{i}")
                        for i, w_ in enumerate(vcols)
                    ]
                y16 = sb.tile([F, rpc], bf16, tag="y16")
                # hop: y_k^T = (G @ y_{k-1})^T, one sweep per part so
                # partial all-gathers overlap the remaining sweeps
                for i, (s, l) in enumerate(fchunks):
                    # loads in consumption order: per part, the hop-1
                    # pin chunk for this sweep, then the streamed batch
                    sbt = {}
                    for pi in range(nfc):
                        if k == 1:
                            pin_load(pi, i)
                        t = gsp.tile(
                            [P, ncores * 512], bf16, tag="gs", name="gs"
                        )
                        nc.sync.dma_start(
                            t[:, 0 : ncores * l].rearrange(
                                "p (c f) -> p c f", c=ncores
                            ),
                            g_str[
                                pi * ncores * P : (pi + 1) * ncores * P,
                                s : s + l,
                            ].rearrange("(c p) f -> p c f", p=P),
                        )
                        sbt[pi] = t
                    hp = ps_hop.tile([F, l], f32, tag=f"hop{i}", name=f"hp{i}")
                    for jn, j in enumerate(jorder):
                        c, m = j // mc, j % mc
                        pi, ml = m2part[m]
                        if m in STR_MS:
                            g = sbt[pi][:, c * l : (c + 1) * l]
                        else:
                            a = c * len(PIN_MS[pi]) + PIN_MS[pi].index(m)
                            g = pin[(pi, i)][:, a * l : (a + 1) * l]
                        nc.tensor.matmul(
                            hp[:], lhsT=v_of(v_cur, j), rhs=g,
                            start=(jn == 0), stop=(jn == jc - 1),
                        )
                    # sweep epilogue: PSUM -> bf16 y16, Wp contribution
                    nc.vector.tensor_copy(y16[:, s : s + l], hp[:])
                    pw = ps_w.tile([F, l], f32, tag="pw")
                    nc.tensor.matmul(
                        pw[:], lhsT=w16[:, k * F : (k + 1) * F],
                        rhs=y16[:, s : s + l], start=True, stop=True,
                    )
                    nc.vector.tensor_add(
                        out_sb[:, s : s + l], out_sb[:, s : s + l], pw[:]
                    )
                    if k < ORDER - 1:
                        # transpose this sweep's rows to natural layout,
                        # partial all-gather; reload rides the scalar
                        # queue so the CC queue stays gather-only
                        m0, nm = parts[i]
                        stage = sb.tile(
                            [P, nm * F], bf16, tag=f"stage{i}",
                            name=f"stage{i}",
                        )
                        for mm in range(nm):
                            m = m0 + mm
                            tp = ps_tp.tile([P, F], bf16, tag="tp", name="tp")
                            nc.tensor.transpose(
                                tp[:], y16[:, m * P : (m + 1) * P],
                                ident16[:],
                            )
                            nc.vector.tensor_copy(
                                stage[:, mm * F : (mm + 1) * F], tp[:]
                            )
                        cc_in = dram.tile(
                            [P, nm * F], bf16, tag=f"ccin{i}", name=f"ccin{i}"
                        )
                        cc_out = dram.tile(
                            [ncores * P, nm * F], bf16, tag=f"ccout{i}",
                            name=f"ccout{i}",
                        )
                        nc.scalar.dma_start(cc_in[:], stage[:])
                        nc.gpsimd.collective_compute(
                            "AllGather",
                            mybir.AluOpType.bypass,
                            replica_groups=[list(range(ncores))],
                            ins=[cc_in.opt()],
                            outs=[cc_out.opt()],
                        )
                        nc.scalar.dma_start(
                            v_next[i][:].rearrange(
                                "p (c m) -> p c m", c=ncores
                            ),
                            cc_out[:].rearrange("(c p) m -> p c m", p=P),
                        )
                if k < ORDER - 1:
                    v_parts = v_next

            nc.scalar.dma_start(out_t, out_sb[:])

    nc.compile()
    return nc


def get_nc(np_total=NP, ncores=NCORES):
    key = (np_total, ncores)
    if key not in _CACHE:
        _CACHE[key] = _build(np_total, ncores)
    return _CACHE[key]


def prep_inputs(x, gso, weight, np_total=NP, ncores=NCORES):
    """Host-side shard prep. Returns in_maps for run_bass_kernel_spmd."""
    import ml_dtypes

    bf = ml_dtypes.bfloat16
    n = x.shape[0]
    rpc = np_total // ncores
    mc = rpc // P

    x = np.asarray(x, dtype=np.float32)
    gso = np.asarray(gso, dtype=np.float32)
    weight = np.asarray(weight, dtype=np.float32)

    wp = np.concatenate(
        [
            weight[0] - weight[2],
            weight[1] - 3.0 * weight[3],
            2.0 * weight[2],
            4.0 * weight[3],
        ],
        axis=1,
    ).astype(np.float32)  # [F, ORDER*F]

    xpad = np.zeros((np_total, F), dtype=np.float32)
    xpad[:n] = x
    gpad = np.zeros((np_total, np_total), dtype=np.float32)
    gpad[:n, :n] = gso
    g16 = gpad.astype(bf)
    x16 = xpad.astype(bf)

    def part_x(ms):
        return (
            x16.reshape(ncores, mc, P, F)[:, ms]
            .transpose(2, 0, 1, 3)
            .reshape(P, ncores * len(ms) * F)
        )

    xv = np.ascontiguousarray(np.concatenate([part_x(ms) for ms in PART_MS], 1))

    in_maps = []
    for c in range(ncores):
        rows = slice(c * rpc, (c + 1) * rpc)
        gt = np.ascontiguousarray(g16[rows, :].T)  # [np_total, rpc]
        # permute output columns to sweep order
        gt = gt.reshape(np_total, mc, P)[:, NEW_MS].reshape(np_total, rpc)
        gt4 = gt.reshape(ncores, mc, P, rpc)
        pin_rows = np.concatenate(
            [
                gt4[cb, m]
                for ms in PIN_MS
                for cb in range(ncores)
                for m in ms
            ],
            axis=0,
        )
        str_rows = np.concatenate(
            [gt4[cb, m] for m in STR_MS for cb in range(ncores)], axis=0
        )
        xtc = np.ascontiguousarray(xpad[rows, :].T)  # [F, rpc] fp32
        xtc = np.ascontiguousarray(
            xtc.reshape(F, mc, P)[:, NEW_MS].reshape(F, rpc)
        )
        in_maps.append(
            {
                "g_pin": np.ascontiguousarray(pin_rows),
                "g_str": np.ascontiguousarray(str_rows),
                "xv": xv,
                "wp": wp,
                "xt": xtc,
            }
        )
    return in_maps


def assemble_output(results, n=N, ncores=NCORES):
    inv = np.argsort(NEW_MS)
    outs = []
    for c in range(ncores):
        o = results[c]["outT"]  # [F, RPC] permuted cols
        outs.append(o.reshape(F, MC, P)[:, inv].reshape(F, RPC))
    out_t = np.concatenate(outs, axis=1)
    return np.ascontiguousarray(out_t.T[:n]).astype(np.float32)


def kernel(x, gso, weight):
    import time

    from concourse import bass_utils

    nc = get_nc()
    in_maps = prep_inputs(x, gso, weight)
    last_err = None
    for attempt in range(3):
        try:
            res = bass_utils.run_bass_kernel_spmd(
                nc, in_maps, core_ids=list(range(NCORES))
            )
            return assemble_output(res.results)
        except Exception as e:  # transient device wedge: retry
            last_err = e
            time.sleep(5.0 * (attempt + 1))
    raise last_err


# revision 7
# speedup vs baseline: 2.2287x; 1.1074x over previous
"""ChebConv (order-4) GNN layer on 8 Trainium2 NeuronCores.

Reference computation (fp32):
    T0 = x, T1 = G x, Tk = 2 G T{k-1} - T{k-2}
    out = sum_k Tk @ W[k]          # [N, F] with N=10000, F=32
Rewritten in the power basis: y0 = x, yk = G y{k-1},
    out = sum_k yk @ Wp[k]  with
    Wp = [W0 - W2, W1 - 3 W3, 2 W2, 4 W3]   (exact modulo fp reassociation)

Strategy (v5):
  * G, the per-hop node features, and Wp[1:] are plain bf16 with fp32
    PSUM accumulation (rel-err ~4e-3 vs the 2e-2 gate); this halves HBM
    bytes and triples PE throughput vs the original hi/lo-split kernel.
  * Row-shard G over 8 cores (1280 padded cols of G^T each, pad
    10000 -> 10240). Per core, 56 of the 80 128-row j-chunks of the
    G^T slice (~18 MB bf16) are pinned in SBUF during hop 1 and reused
    by hops 2-3, which then stream only ~6.5 MB each: hop 1 runs at
    the HBM roofline (~75 us), hops 2-3 at the PE roofline (~45 us).
  * HWDGE trigger instructions cost ~0.6 us on the issuing engine and
    block on descriptor backpressure, so the host lays G out as
    partition-major per-sweep images: every pinned-set load is ONE
    plain 2D DMA with multi-KB per-partition descriptors (9 triggers
    for all of hop 1's pins), and the 24 streamed chunks load as one
    2D batch per (sweep, part) into a 4-deep ring.
  * Sweeps run in gather-part order [2-chunk part, 4, 4] (host permutes
    G^T/x^T columns so sweep columns stay contiguous). Hops 2-3 fire a
    partial AllGather per sweep, and each hop consumes j-chunks in the
    same part order, so hop k+1 starts as soon as hop k's first
    (smallest, earliest) gather lands. Hop 1's gathers are instead
    DEFERRED and merged into a single AllGather of all of y1 at hop
    end: collectives execute serially on the CC cores behind a
    ~40-85 us one-time init barrier (starts ~21 us into the NEFF), and
    any collective executing while hop 1 still streams G starves the
    HWDGE drain and convoys the whole hop. A tiny dummy AllGather
    issued first absorbs the first-call warmup during hop 1's tail.
    Reloads of gathered y into per-part v tiles ride the scalar queue
    so the CC queue runs gathers back-to-back.
  * Each hop computes y_k^T in 3 sweeps (one PSUM bank per sweep): per
    j-chunk one bf16 matmul (lhsT = v[j] [128,32], rhs = G^T tile
    [128,<=512]) accumulates over all 80 chunks; the epilogue copies
    PSUM to bf16 y16, adds the bf16 Wp_k term into the fp32 output
    accumulator (bf16 rhs streams at full PE rate; fp32 runs LOW_HIGH
    at half rate), PE-transposes the sweep rows and stages the gather
    input. The k=0 term uses the host fp32 xT slice.
  * Output is returned transposed and column-permuted ([32, 1280] per
    core); the host concatenates, un-permutes, transposes and drops
    padding.
"""

import sys

if "/opt/trn_rl_repo" not in sys.path:
    sys.path.insert(0, "/opt/trn_rl_repo")

import numpy as np

N = 10000
F = 32
ORDER = 4
NCORES = 8
P = 128
NP = 10240  # padded node count: divisible by NCORES * P
RPC = NP // NCORES  # rows per core (1280)
JC = NP // P  # global 128-row chunks (80)
MC = RPC // P  # local 128-row chunks per core (10)

# gather parts in sweep order; per part: natural m-chunks (host permutes
# columns to this order), pinned m-chunks, streamed m-chunk
PART_MS = [[8, 9], [0, 1, 2, 3], [4, 5, 6, 7]]
PIN_MS = [[8], [0, 1, 2], [4, 5, 6]]
STR_MS = [9, 3, 7]
NEW_MS = [m for ms in PART_MS for m in ms]  # host column permutation

_CACHE = {}


def _build(np_total, ncores):
    from concourse import bacc, masks, mybir, tile

    rpc = np_total // ncores
    jc = np_total // P
    mc = rpc // P
    f32 = mybir.dt.float32
    bf16 = mybir.dt.bfloat16
    nfc = len(PART_MS)
    parts = []
    s = 0
    for ms in PART_MS:
        parts.append((s // P, len(ms)))
        s += len(ms) * P
    fchunks = [(m0 * P, nm * P) for (m0, nm) in parts]
    vcols = [ncores * nm * F for (m0, nm) in parts]
    # stage/v column offset of each part (units of F cols)
    part_off = [0, 0, 0]
    for pi in range(1, nfc):
        part_off[pi] = part_off[pi - 1] + len(PART_MS[pi - 1])

    nc = bacc.Bacc(
        "TRN2", target_bir_lowering=False, debug=False, num_devices=ncores
    )
    # pinned G, one partition-major image per sweep: row p holds, for
    # each part pi then each (c, m-in-run) a, that chunk's sweep-i
    # column slice: [P, 56 * l_i]
    g_pins = [
        nc.dram_tensor(
            f"g_pin{i}", [P, ncores * 7 * l], bf16, kind="ExternalInput"
        ).ap()
        for i, (s, l) in enumerate(fchunks)
    ]
    # streamed G, same layout: [P, 24 * l_i], parts-major
    g_strs = [
        nc.dram_tensor(
            f"g_str{i}", [P, ncores * 3 * l], bf16, kind="ExternalInput"
        ).ap()
        for i, (s, l) in enumerate(fchunks)
    ]
    # column offset (elements) of part pi inside g_pins[i] / g_strs[i]
    pin_coff = [0, 0, 0]
    str_coff = [0, 0, 0]
    for pi in range(1, nfc):
        pin_coff[pi] = pin_coff[pi - 1] + ncores * len(PIN_MS[pi - 1])
        str_coff[pi] = str_coff[pi - 1] + ncores

    xv = nc.dram_tensor("xv", [P, sum(vcols)], bf16, kind="ExternalInput").ap()
    xt = nc.dram_tensor("xt", [F, rpc], f32, kind="ExternalInput").ap()
    wp = nc.dram_tensor("wp", [F, ORDER * F], f32, kind="ExternalInput").ap()
    out_t = nc.dram_tensor("outT", [F, rpc], f32, kind="ExternalOutput").ap()

    m2part = {}
    for pi, ms in enumerate(PART_MS):
        for ml, m in enumerate(ms):
            m2part[m] = (pi, ml)

    # consumption order: parts in gather-firing order; within a part
    # pinned chunks (c-major), then the streamed batch
    jorder = []
    for pi in range(nfc):
        jorder += [c * mc + m for c in range(ncores) for m in PIN_MS[pi]]
        jorder += [c * mc + STR_MS[pi] for c in range(ncores)]

    with tile.TileContext(nc) as tc:
        with (
            tc.tile_pool(name="const", bufs=1) as constp,
            tc.tile_pool(name="gsp", bufs=4) as gsp,
            tc.tile_pool(name="vp", bufs=2) as vp,
            tc.tile_pool(name="sb", bufs=2) as sb,
            tc.tile_pool(name="ps_hop", bufs=1, space="PSUM") as ps_hop,
            tc.tile_pool(name="ps_tp", bufs=2, space="PSUM") as ps_tp,
            tc.tile_pool(name="ps_w", bufs=2, space="PSUM") as ps_w,
            tc.tile_pool(name="dram", bufs=2, space="DRAM") as dram,
        ):
            ident = constp.tile([P, P], f32)
            masks.make_identity(nc, ident[:])
            xt_sb = constp.tile([F, rpc], f32)
            nc.scalar.dma_start(xt_sb[:], xt)
            w_sb = constp.tile([F, ORDER * F], f32)
            nc.scalar.dma_start(w_sb[:], wp)
            out_sb = constp.tile([F, rpc], f32)
            ident16 = constp.tile([F, F], bf16)
            nc.vector.tensor_copy(ident16[:], ident[0:F, 0:F])
            w16 = constp.tile([F, ORDER * F], bf16)
            nc.vector.tensor_copy(w16[:], w_sb[:])

            # tiny dummy collective: pays the first-collective warmup
            # (on top of the CC init barrier) off the critical path
            d_in = dram.tile([P, F], bf16, tag="d_in", name="d_in")
            d_out = dram.tile([P * ncores, F], bf16, tag="d_out", name="d_out")
            d_sb = constp.tile([P, F], bf16)
            nc.vector.tensor_copy(d_sb[:], ident[:, 0:F])
            nc.scalar.dma_start(d_in[:], d_sb[:])
            nc.gpsimd.collective_compute(
                "AllGather",
                mybir.AluOpType.bypass,
                replica_groups=[list(range(ncores))],
                ins=[d_in.opt()],
                outs=[d_out.opt()],
            )

            # pinned G: one tile per (part, sweep), one 2D DMA each
            pin = {}
            for pi in range(nfc):
                na = ncores * len(PIN_MS[pi])
                for i, (s, l) in enumerate(fchunks):
                    pin[(pi, i)] = constp.tile(
                        [P, na * l], bf16, name=f"pin{pi}_{i}"
                    )

            # v holds y_{k-1} as bf16, one tile per part so next-hop
            # matmuls only depend on the gather that produced them
            v_parts = []
            off = 0
            for i, w_ in enumerate(vcols):
                vt = vp.tile([P, w_], bf16, tag=f"v{i}", name=f"v{i}")
                nc.scalar.dma_start(vt[:], xv[:, off : off + w_])
                off += w_
                v_parts.append(vt)

            def v_of(vps, j):
                c, m = j // mc, j % mc
                pi, ml = m2part[m]
                nm = len(PART_MS[pi])
                col = (c * nm + ml) * F
                return vps[pi][:, col : col + F]

            # k = 0 contribution: out^T = Wp_0^T @ x^T (pure fp32)
            for s, l in fchunks:
                pw = ps_w.tile([F, l], f32, tag="pw")
                nc.tensor.matmul(
                    pw[:], lhsT=w_sb[:, 0:F], rhs=xt_sb[:, s : s + l],
                    start=True, stop=True,
                )
                nc.vector.tensor_copy(out_sb[:, s : s + l], pw[:])

            def all_gather(cc_in_src, nmtot, tag):
                cc_in = dram.tile(
                    [P, nmtot * F], bf16, tag=f"ci{tag}", name=f"ci{tag}"
                )
                cc_out = dram.tile(
                    [ncores * P, nmtot * F], bf16, tag=f"co{tag}",
                    name=f"co{tag}",
                )
                nc.scalar.dma_start(cc_in[:], cc_in_src)
                nc.gpsimd.collective_compute(
                    "AllGather",
                    mybir.AluOpType.bypass,
                    replica_groups=[list(range(ncores))],
                    ins=[cc_in.opt()],
                    outs=[cc_out.opt()],
                )
                return cc_out

            def reload(cc_out, col0, nm, v_dst):
                # v part reload on the scalar queue (CC queue stays
                # gather-only; sync queue stays G-only)
                nc.scalar.dma_start(
                    v_dst[:].rearrange("p (c m) -> p c m", c=ncores),
                    cc_out[:, col0 * F : (col0 + nm) * F].rearrange(
                        "(c p) m -> p c m", p=P
                    ),
                )

            for k in range(1, ORDER):
                v_cur = v_parts
                if k < ORDER - 1:
                    v_next = [
                        vp.tile([P, w_], bf16, tag=f"v{i}", name=f"vn{i}")
                        for i, w_ in enumerate(vcols)
                    ]
                y16 = sb.tile([F, rpc], bf16, tag="y16")
                if k == 1:
                    stage_full = sb.tile([P, mc * F], bf16, tag="stF")
                for i, (s, l) in enumerate(fchunks):
                    # loads in consumption order per part: hop-1 pin
                    # image chunk, then the streamed batch (all 2D)
                    sbt = {}
                    for pi in range(nfc):
                        if k == 1:
                            na = ncores * len(PIN_MS[pi])
                            nc.sync.dma_start(
                                pin[(pi, i)][:],
                                g_pins[i][:, pin_coff[pi] * l : (pin_coff[pi] + na) * l],
                            )
                        t = gsp.tile(
                            [P, ncores * 512], bf16, tag="gs", name="gs"
                        )
                        nc.sync.dma_start(
                            t[:, 0 : ncores * l],
                            g_strs[i][:, str_coff[pi] * l : (str_coff[pi] + ncores) * l],
                        )
                        sbt[pi] = t
                    hp = ps_hop.tile([F, l], f32, tag=f"hop{i}", name=f"hp{i}")
                    for jn, j in enumerate(jorder):
                        c, m = j // mc, j % mc
                        pi, ml = m2part[m]
                        if m in STR_MS:
                            g = sbt[pi][:, c * l : (c + 1) * l]
                        else:
                            a = c * len(PIN_MS[pi]) + PIN_MS[pi].index(m)
                            g = pin[(pi, i)][:, a * l : (a + 1) * l]
                        nc.tensor.matmul(
                            hp[:], lhsT=v_of(v_cur, j), rhs=g,
                            start=(jn == 0), stop=(jn == jc - 1),
                        )
                    # sweep epilogue: PSUM -> bf16 y16, Wp contribution
                    nc.vector.tensor_copy(y16[:, s : s + l], hp[:])
                    pw = ps_w.tile([F, l], f32, tag="pw")
                    nc.tensor.matmul(
                        pw[:], lhsT=w16[:, k * F : (k + 1) * F],
                        rhs=y16[:, s : s + l], start=True, stop=True,
                    )
                    nc.vector.tensor_add(
                        out_sb[:, s : s + l], out_sb[:, s : s + l], pw[:]
                    )
                    if k < ORDER - 1:
                        # transpose sweep rows to natural layout
                        m0, nm = parts[i]
                        if k == 1:
                            stage = stage_full[
                                :, part_off[i] * F : (part_off[i] + nm) * F
                            ]
                        else:
                            st = sb.tile(
                                [P, nm * F], bf16, tag=f"stage{i}",
                                name=f"stage{i}",
                            )
                            stage = st[:]
                        for mm in range(nm):
                            m = m0 + mm
                            tp = ps_tp.tile([P, F], bf16, tag="tp", name="tp")
                            nc.tensor.transpose(
                                tp[:], y16[:, m * P : (m + 1) * P],
                                ident16[:],
                            )
                            nc.vector.tensor_copy(
                                stage[:, mm * F : (mm + 1) * F], tp[:]
                            )
                        if k > 1:
                            # hops 2+: partial gather per sweep,
                            # overlapping the remaining sweeps
                            cc_out = all_gather(stage, nm, f"p{i}")
                            reload(cc_out, 0, nm, v_next[i])
                if k == 1:
                    # hop 1: single deferred gather of all of y1 at hop
                    # end - a collective executing mid-hop-1 would
                    # starve the HWDGE drain and convoy the G stream
                    cc_out = all_gather(stage_full[:], mc, "h1")
                    for pi in range(nfc):
                        reload(
                            cc_out, part_off[pi], len(PART_MS[pi]), v_next[pi]
                        )
                if k < ORDER - 1:
                    v_parts = v_next

            nc.scalar.dma_start(out_t, out_sb[:])

    nc.compile()
    return nc


def get_nc(np_total=NP, ncores=NCORES):
    key = (np_total, ncores)
    if key not in _CACHE:
        _CACHE[key] = _build(np_total, ncores)
    return _CACHE[key]


def prep_inputs(x, gso, weight, np_total=NP, ncores=NCORES):
    """Host-side shard prep. Returns in_maps for run_bass_kernel_spmd."""
    import ml_dtypes

    bf = ml_dtypes.bfloat16
    n = x.shape[0]
    rpc = np_total // ncores
    mc = rpc // P

    x = np.asarray(x, dtype=np.float32)
    gso = np.asarray(gso, dtype=np.float32)
    weight = np.asarray(weight, dtype=np.float32)

    wp = np.concatenate(
        [
            weight[0] - weight[2],
            weight[1] - 3.0 * weight[3],
            2.0 * weight[2],
            4.0 * weight[3],
        ],
        axis=1,
    ).astype(np.float32)  # [F, ORDER*F]

    xpad = np.zeros((np_total, F), dtype=np.float32)
    xpad[:n] = x
    gpad = np.zeros((np_total, np_total), dtype=np.float32)
    gpad[:n, :n] = gso
    g16 = gpad.astype(bf)
    x16 = xpad.astype(bf)

    def part_x(ms):
        return (
            x16.reshape(ncores, mc, P, F)[:, ms]
            .transpose(2, 0, 1, 3)
            .reshape(P, ncores * len(ms) * F)
        )

    xv = np.ascontiguousarray(np.concatenate([part_x(ms) for ms in PART_MS], 1))

    fchunks = []
    s = 0
    for ms in PART_MS:
        fchunks.append((s, len(ms) * P))
        s += len(ms) * P

    in_maps = []
    for c in range(ncores):
        rows = slice(c * rpc, (c + 1) * rpc)
        gt = np.ascontiguousarray(g16[rows, :].T)  # [np_total, rpc]
        # permute output columns to sweep order
        gt = gt.reshape(np_total, mc, P)[:, NEW_MS].reshape(np_total, rpc)
        gt4 = gt.reshape(ncores, mc, P, rpc)
        # partition-major per-sweep images: [P, chunks * l]
        pin_rows = np.stack(
            [gt4[cb, m] for ms in PIN_MS for cb in range(ncores) for m in ms]
        )  # [56, P, rpc]
        str_rows = np.stack(
            [gt4[cb, m] for m in STR_MS for cb in range(ncores)]
        )  # [24, P, rpc]
        m = {"xv": xv, "wp": wp}
        for i, (s, l) in enumerate(fchunks):
            m[f"g_pin{i}"] = np.ascontiguousarray(
                pin_rows[:, :, s : s + l].transpose(1, 0, 2).reshape(P, -1)
            )
            m[f"g_str{i}"] = np.ascontiguousarray(
                str_rows[:, :, s : s + l].transpose(1, 0, 2).reshape(P, -1)
            )
        xtc = np.ascontiguousarray(xpad[rows, :].T)  # [F, rpc] fp32
        m["xt"] = np.ascontiguousarray(
            xtc.reshape(F, mc, P)[:, NEW_MS].reshape(F, rpc)
        )
        in_maps.append(m)
    return in_maps


def assemble_output(results, n=N, ncores=NCORES):
    inv = np.argsort(NEW_MS)
    outs = []
    for c in range(ncores):
        o = results[c]["outT"]  # [F, RPC] permuted cols
        outs.append(o.reshape(F, MC, P)[:, inv].reshape(F, RPC))
    out_t = np.concatenate(outs, axis=1)
    return np.ascontiguousarray(out_t.T[:n]).astype(np.float32)


def kernel(x, gso, weight):
    import time

    from concourse import bass_utils

    nc = get_nc()
    in_maps = prep_inputs(x, gso, weight)
    last_err = None
    for attempt in range(3):
        try:
            res = bass_utils.run_bass_kernel_spmd(
                nc, in_maps, core_ids=list(range(NCORES))
            )
            return assemble_output(res.results)
        except Exception as e:  # transient device wedge: retry
            last_err = e
            time.sleep(5.0 * (attempt + 1))
    raise last_err
